# revision 1
# baseline (speedup 1.0000x reference)
"""MinkUNet stem+stage1 on 8 Trainium2 NeuronCores.

Strategy (data-parallel over voxel rows, per sharding hint):
  - Host: im2col index gathers (nbr tables are kernel inputs), sharding,
    chunk-transposed G layout so the device PE needs no transposes.
  - Device (SPMD x8, via bass/Tile): sparse-conv as K-chunked accumulating
    matmuls, BatchNorm stats with an 8-core AllReduce, affine+ReLU,
    residual adds. One launch per conv layer; 5 distinct compiled programs.
"""
import numpy as np

import concourse.bacc as bacc
import concourse.mybir as mybir
import concourse.tile as tile
from concourse.bass_utils import run_bass_kernel_spmd

NCORES = 8
P = 128
C = 32
BLK = 512
FP32 = mybir.dt.float32
EPS = 1e-5

_prog_cache = {}


def _build_layer(rows_pc, nchunks, cin_cols, residual, inv_n):
    """One conv+BN(+residual)+ReLU layer program for all 8 cores.

    rows_pc:  output rows per core (multiple of BLK)
    nchunks:  K chunks of 128 (ceil(27*cin/128))
    residual: add x1 before final relu
    inv_n:    1/N_true for BN statistics
    """
    nb = rows_pc // BLK
    ycols = (nb + 3) // 4 * BLK          # grouped layout columns
    nc = bacc.Bacc("TRN2", target_bir_lowering=False)
    gt = nc.dram_tensor("gt", [nchunks, P, rows_pc], FP32, kind="ExternalInput")
    w = nc.dram_tensor("w", [nchunks, P, C], FP32, kind="ExternalInput")
    gamma = nc.dram_tensor("gamma", [C, 1], FP32, kind="ExternalInput")
    beta = nc.dram_tensor("beta", [C, 1], FP32, kind="ExternalInput")
    if residual:
        x1 = nc.dram_tensor("x1", [P, ycols], FP32, kind="ExternalInput")
    y = nc.dram_tensor("y", [P, ycols], FP32, kind="ExternalOutput")

    with tile.TileContext(nc) as tc:
        with (
            tc.tile_pool(name="sb", bufs=2) as sb,
            tc.tile_pool(name="sb1", bufs=1) as sb1,
            tc.tile_pool(name="ps", bufs=2, space="PSUM") as ps,
            tc.tile_pool(name="dram", bufs=1, space="DRAM") as dram,
        ):
            w_t = sb1.tile([P, nchunks, C], FP32, name="w_t")
            nc.sync.dma_start(w_t[:], w[:].rearrange("n p c -> p n c"))
            gam_t = sb1.tile([C, 1], FP32, name="gam_t")
            nc.sync.dma_start(gam_t[:], gamma[:])
            bet_t = sb1.tile([C, 1], FP32, name="bet_t")
            nc.sync.dma_start(bet_t[:], beta[:])

            raw = sb1.tile([P, ycols], FP32, name="raw")
            stats = sb1.tile([P, 2 * ((nb + 3) // 4)], FP32, name="stats")
            nc.gpsimd.memset(stats[:], 0.0)

            # pass 1: conv + per-block partial stats
            for b in range(nb):
                g = b % 4
                col = (b // 4) * BLK
                sl = slice(32 * g, 32 * g + 32)
                gtile = sb.tile([P, nchunks, BLK], FP32, name="gtile", tag="gtile")
                nc.sync.dma_start(gtile[:], gt[:, :, b * BLK:(b + 1) * BLK].rearrange("n p c -> p n c"))
                acc = ps.tile([P, BLK], FP32, name="acc", tag="acc")
                for cch in range(nchunks):
                    nc.tensor.matmul(
                        acc[sl, :], w_t[:, cch, :], gtile[:, cch, :],
                        start=(cch == 0), stop=(cch == nchunks - 1),
                        tile_position=(0, 32 * g),
                    )
                nc.vector.tensor_copy(raw[sl, col:col + BLK], acc[sl, :])
                sqg = sb.tile([P, BLK], FP32, name="sqg", tag="sqg")
                nc.vector.tensor_tensor(
                    out=sqg[sl, :], in0=raw[sl, col:col + BLK],
                    in1=raw[sl, col:col + BLK], op=mybir.AluOpType.mult)
                nc.vector.tensor_reduce(
                    stats[sl, 2 * (b // 4):2 * (b // 4) + 1],
                    raw[sl, col:col + BLK],
                    axis=mybir.AxisListType.X, op=mybir.AluOpType.add)
                nc.vector.tensor_reduce(
                    stats[sl, 2 * (b // 4) + 1:2 * (b // 4) + 2],
                    sqg[sl, :],
                    axis=mybir.AxisListType.X, op=mybir.AluOpType.add)

            # fold stats: free-axis reduce then cross-group collect
            part = sb1.tile([P, 2], FP32, name="part")
            nc.vector.tensor_reduce(part[:, 0:1], stats[:].rearrange("p (n t) -> p t n", t=2)[:, 0, :],
                                    axis=mybir.AxisListType.X, op=mybir.AluOpType.add)
            nc.vector.tensor_reduce(part[:, 1:2], stats[:].rearrange("p (n t) -> p t n", t=2)[:, 1, :],
                                    axis=mybir.AxisListType.X, op=mybir.AluOpType.add)
            stage = sb1.tile([C, 8], FP32, name="stage")
            nc.vector.tensor_copy(stage[:, 0:2], part[0:C, :])
            for g in range(1, 4):
                nc.sync.dma_start(stage[:, 2 * g:2 * g + 2], part[32 * g:32 * g + 32, :])
            loc = sb1.tile([C, 2], FP32, name="loc")
            nc.vector.tensor_reduce(loc[:, :], stage[:].rearrange("p (g t) -> p t g", t=2),
                                    axis=mybir.AxisListType.X, op=mybir.AluOpType.add)
            cin_d = dram.tile([C, 2], FP32, name="cin_d")
            cout_d = dram.tile([C, 2], FP32, name="cout_d")
            nc.sync.dma_start(cin_d[:], loc[:])
            nc.gpsimd.collective_compute(
                "AllReduce", mybir.AluOpType.add,
                replica_groups=[list(range(NCORES))],
                ins=[cin_d.opt()], outs=[cout_d.opt()],
            )
            tot = sb1.tile([C, 2], FP32, name="tot")
            nc.sync.dma_start(tot[:], cout_d[:])

            # s = gamma / sqrt(var+eps); bb = beta - mu*s
            mu = sb1.tile([C, 1], FP32, name="mu")
            nc.vector.tensor_scalar_mul(mu[:], tot[:, 0:1], float(inv_n))
            var = sb1.tile([C, 1], FP32, name="var")
            nc.vector.tensor_scalar_mul(var[:], tot[:, 1:2], float(inv_n))
            mu2 = sb1.tile([C, 1], FP32, name="mu2")
            nc.vector.tensor_tensor(out=mu2[:], in0=mu[:], in1=mu[:], op=mybir.AluOpType.mult)
            nc.vector.tensor_tensor(out=var[:], in0=var[:], in1=mu2[:], op=mybir.AluOpType.subtract)
            nc.vector.tensor_scalar_add(var[:], var[:], EPS)
            std = sb1.tile([C, 1], FP32, name="std")
            nc.scalar.sqrt(std[:], var[:])
            rstd = sb1.tile([C, 1], FP32, name="rstd")
            nc.vector.reciprocal(rstd[:], std[:])
            s_v = sb1.tile([P, 1], FP32, name="s_v")
            b_v = sb1.tile([P, 1], FP32, name="b_v")
            nc.vector.tensor_tensor(out=s_v[0:C, :], in0=gam_t[:], in1=rstd[:], op=mybir.AluOpType.mult)
            mus = sb1.tile([C, 1], FP32, name="mus")
            nc.vector.tensor_tensor(out=mus[:], in0=mu[:], in1=s_v[0:C, :], op=mybir.AluOpType.mult)
            nc.vector.tensor_tensor(out=b_v[0:C, :], in0=bet_t[:], in1=mus[:], op=mybir.AluOpType.subtract)
            for g in range(1, 4):
                nc.sync.dma_start(s_v[32 * g:32 * g + 32, :], s_v[0:C, :])
                nc.sync.dma_start(b_v[32 * g:32 * g + 32, :], b_v[0:C, :])

            # pass 2: affine (+residual) + relu, write out
            if residual:
                x1_t = sb1.tile([P, ycols], FP32, name="x1_t")
                nc.sync.dma_start(x1_t[:], x1[:])
            out_t = sb1.tile([P, ycols], FP32, name="out_t")
            for b in range(nb):
                g = b % 4
                col = (b // 4) * BLK
                sl = slice(32 * g, 32 * g + 32)
                tmp = sb.tile([P, BLK], FP32, name="tmp", tag="tmp")
                nc.vector.tensor_scalar(
                    out=tmp[sl, :], in0=raw[sl, col:col + BLK],
                    scalar1=s_v[sl, :], scalar2=b_v[sl, :],
                    op0=mybir.AluOpType.mult, op1=mybir.AluOpType.add)
                if residual:
                    nc.vector.tensor_tensor(
                        out=tmp[sl, :], in0=tmp[sl, :],
                        in1=x1_t[sl, col:col + BLK], op=mybir.AluOpType.add)
                nc.scalar.activation(out_t[sl, col:col + BLK], tmp[sl, :],
                                     mybir.ActivationFunctionType.Relu)
            nc.sync.dma_start(y[:], out_t[:])
    nc.compile()
    return nc, ycols


def _warmup(prog, ycols, rows_pc, nchunks, residual):
    m = {"gt": np.zeros((nchunks, P, rows_pc), np.float32),
         "w": np.zeros((nchunks, P, C), np.float32),
         "gamma": np.ones((C, 1), np.float32),
         "beta": np.zeros((C, 1), np.float32)}
    if residual:
        m["x1"] = np.zeros((P, ycols), np.float32)
    run_bass_kernel_spmd(prog, [m] * NCORES, core_ids=list(range(NCORES)))


def _get_prog(key, *args):
    if key not in _prog_cache:
        import time
        t0 = time.time()
        prog, ycols = _build_layer(*args)
        _warmup(prog, ycols, args[0], args[1], args[3])
        kernel.compile_s += time.time() - t0
        _prog_cache[key] = (prog, ycols)
    return _prog_cache[key]


def _host_gather_gt(feat, nbrT, rows_pc, nchunks, cin):
    """feat [N, cin]; nbrT [rows_total, K] -> per-core G_T [nchunks,128,rows_pc]."""
    K = nbrT.shape[1]
    q = K * cin
    outs = []
    for c in range(NCORES):
        sl = nbrT[c * rows_pc:(c + 1) * rows_pc]
        n = sl.shape[0]
        g = np.zeros((rows_pc, nchunks * P), np.float32)
        if n:
            valid = sl >= 0
            gg = feat[np.clip(sl, 0, None)]          # [n, K, cin]
            gg[~valid] = 0.0
            g[:n, :q] = gg.reshape(n, q)
        outs.append(np.ascontiguousarray(g.reshape(rows_pc, nchunks, P).transpose(1, 2, 0)))
    return outs


def _decode(y_parts, rows_pc, rows_true_total, ycols):
    """y core parts [128, ycols] grouped -> full [rows, 32]."""
    nb = rows_pc // BLK
    full = np.empty((NCORES * rows_pc, C), np.float32)
    for ci, yp in enumerate(y_parts):
        for b in range(nb):
            g = b % 4
            col = (b // 4) * BLK
            blkv = yp[32 * g:32 * g + 32, col:col + BLK]     # [32, 512]
            r0 = ci * rows_pc + b * BLK
            full[r0:r0 + BLK] = blkv.T
    return full[:rows_true_total]


def _encode(x, rows_pc, ycols):
    """full [NCORES*rows_pc(padded ok), 32] -> per-core [128, ycols] grouped."""
    nb = rows_pc // BLK
    need = NCORES * rows_pc
    if x.shape[0] < need:
        x = np.concatenate([x, np.zeros((need - x.shape[0], C), np.float32)])
    outs = []
    for ci in range(NCORES):
        yp = np.zeros((P, ycols), np.float32)
        for b in range(nb):
            g = b % 4
            col = (b // 4) * BLK
            r0 = ci * rows_pc + b * BLK
            yp[32 * g:32 * g + 32, col:col + BLK] = x[r0:r0 + BLK].T
        outs.append(yp)
    return outs


def _run_layer(key, feat, nbrT, W, gamma, beta, n_true, residual_x=None):
    """One conv(+bn+relu / +residual) layer on the 8 cores."""
    rows_total = nbrT.shape[0]
    K, cin = W.shape[0], W.shape[1]
    rows_pc = -(-rows_total // (NCORES * BLK)) * BLK
    nchunks = -(-(K * cin) // P)
    prog, ycols = _get_prog((rows_pc, nchunks, residual_x is not None, n_true),
                            rows_pc, nchunks, cin, residual_x is not None,
                            1.0 / n_true)
    import time as _t
    _tg = _t.time()
    nbrT_pad = np.full((NCORES * rows_pc, K), -1, np.int32)
    nbrT_pad[:rows_total] = nbrT
    gts = _host_gather_gt(feat, nbrT_pad, rows_pc, nchunks, cin)
    w_pad = np.zeros((nchunks, P, C), np.float32)
    w_flat = W.reshape(K * cin, C)
    for c in range(nchunks):
        lo = c * P
        hi = min((c + 1) * P, K * cin)
        w_pad[c, :hi - lo] = w_flat[lo:hi]
    g1 = np.ascontiguousarray(gamma.reshape(C, 1).astype(np.float32))
    b1 = np.ascontiguousarray(beta.reshape(C, 1).astype(np.float32))
    if residual_x is not None:
        x1s = _encode(residual_x, rows_pc, ycols)
    in_maps = []
    for ci in range(NCORES):
        m = {"gt": gts[ci], "w": w_pad, "gamma": g1, "beta": b1}
        if residual_x is not None:
            m["x1"] = x1s[ci]
        in_maps.append(m)
    import time
    kernel.host_s += time.time() - _tg
    t0 = time.time()
    res = run_bass_kernel_spmd(prog, in_maps, core_ids=list(range(NCORES)))
    _run_layer.exec_s += time.time() - t0
    y_parts = [res.results[ci]["y"] for ci in range(NCORES)]
    return _decode(y_parts, rows_pc, rows_total, ycols)


_run_layer.exec_s = 0.0


def kernel(voxel_features, W_stem1, W_stem2, W_down, W_r1a, W_r1b, W_r2a, W_r2b,
           gammas, betas, nbr0, down1, nbr1):
    vf = np.asarray(voxel_features, np.float32)
    nbr0T = np.ascontiguousarray(np.asarray(nbr0, np.int32).T)
    down1T = np.ascontiguousarray(np.asarray(down1, np.int32).T)
    nbr1T = np.ascontiguousarray(np.asarray(nbr1, np.int32).T)
    g = np.asarray(gammas, np.float32)
    b = np.asarray(betas, np.float32)
    Ws = [np.asarray(w, np.float32) for w in
          (W_stem1, W_stem2, W_down, W_r1a, W_r1b, W_r2a, W_r2b)]
    N0 = vf.shape[0]
    M1 = down1T.shape[0]
    _run_layer.exec_s = 0.0
    kernel.compile_s = 0.0
    kernel.host_s = 0.0

    x = _run_layer("stem1", vf, nbr0T, Ws[0], g[0], b[0], N0)
    x = _run_layer("stem2", x, nbr0T, Ws[1], g[1], b[1], N0)
    x1 = _run_layer("down", x, down1T, Ws[2], g[2], b[2], M1)
    zero = np.zeros_like(x1)
    h = _run_layer("r1a", x1, nbr1T, Ws[3], g[3], b[3], M1, residual_x=zero)
    x1 = _run_layer("r1b", h, nbr1T, Ws[4], g[4], b[4], M1, residual_x=x1)
    h = _run_layer("r2a", x1, nbr1T, Ws[5], g[5], b[5], M1, residual_x=zero)
    out = _run_layer("r2b", h, nbr1T, Ws[6], g[6], b[6], M1, residual_x=x1)
    kernel.exec_s = _run_layer.exec_s
    return out


kernel.exec_s = 0.0
kernel.compile_s = 0.0
kernel.host_s = 0.0



# revision 2
# speedup vs baseline: 1.0039x; 1.0039x over previous
"""MinkUNet stem+stage1, fully on-device on 8 Trainium2 NeuronCores.

One Bass program runs all 7 sparse-conv layers:
  - features live on device; per-layer AllGather + per-core halo window copy
    (dynamic partition-id offset) keep a local window in gather range
  - sparse gathers via gpsimd dma_gather (int16 window-relative indices,
    invalid entries point at interleaved zero rows)
  - conv = DVE 32x32 block-transpose + per-(k, group) 32x32x512 matmuls
  - BN stats via per-super reduction + 8-core AllReduce
Host only remaps index tables to int16 window layout and reassembles output.
"""
import numpy as np

import concourse.bacc as bacc
import concourse.mybir as mybir
import concourse.tile as tile
import concourse.bass as bass
from concourse.bass import DynSlice
from concourse.bass_utils import run_bass_kernel_spmd
from concourse.zero import tile_zero


def _make_runner(nc, n_cores):
    """Jitted shard_map executor for nc with device-side zero outputs."""
    import jax
    import jax.numpy as jnp
    from jax.sharding import Mesh, PartitionSpec, NamedSharding
    from jax.experimental.shard_map import shard_map
    from concourse import bass2jax, mybir as mb
    bass2jax.install_neuronx_cc_hook()

    partition_name = (nc.partition_id_tensor.name
                      if nc.partition_id_tensor else None)
    in_names, out_names, out_avals = [], [], []
    for alloc in nc.m.functions[0].allocations:
        if not isinstance(alloc, mb.MemoryLocationSet):
            continue
        name = alloc.memorylocations[0].name
        if alloc.kind == "ExternalInput":
            if name != partition_name:
                in_names.append(name)
        elif alloc.kind == "ExternalOutput":
            out_names.append(name)
            out_avals.append(jax.core.ShapedArray(
                tuple(alloc.tensor_shape), mb.dt.np(alloc.dtype)))
    n_params = len(in_names)
    n_outs = len(out_avals)
    all_names = list(in_names) + list(out_names)
    if partition_name is not None:
        all_names.append(partition_name)

    def _body(*args):
        operands = list(args)
        if partition_name is not None:
            operands.append(bass2jax.partition_id_tensor())
        return tuple(bass2jax._bass_exec_p.bind(
            *operands,
            out_avals=tuple(out_avals),
            in_names=tuple(all_names),
            out_names=tuple(out_names),
            lowering_input_output_aliases=(),
            sim_require_finite=True,
            sim_require_nnan=True,
            nc=nc,
        ))

    devices = jax.devices()[:n_cores]
    mesh = Mesh(np.asarray(devices), ("core",))
    sh = NamedSharding(mesh, PartitionSpec("core"))
    in_specs = (PartitionSpec("core"),) * (n_params + n_outs)
    out_specs = (PartitionSpec("core"),) * n_outs
    sharded = jax.jit(
        shard_map(_body, mesh=mesh, in_specs=in_specs, out_specs=out_specs,
                  check_rep=False),
        donate_argnums=tuple(range(n_params, n_params + n_outs)),
        keep_unused=True)
    zeros_fn = jax.jit(
        lambda: tuple(jnp.zeros((n_cores * a.shape[0],) + tuple(a.shape[1:]),
                                a.dtype) for a in out_avals),
        out_shardings=(sh,) * n_outs)

    def run(in_maps, timers):
        import time
        from concurrent.futures import ThreadPoolExecutor
        t0 = time.time()
        zouts = zeros_fn()
        jax.block_until_ready(zouts)

        def put_piece(args):
            i, c = args
            arr = np.ascontiguousarray(np.asarray(in_maps[c][in_names[i]]))
            return i, c, jax.device_put(arr, devices[c])

        pieces = {}
        jobs = [(i, c) for i in range(n_params) for c in range(n_cores)]
        with ThreadPoolExecutor(16) as ex:
            for i, c, a in ex.map(put_piece, jobs):
                pieces[(i, c)] = a
        gin = []
        for i in range(n_params):
            singles = [pieces[(i, c)] for c in range(n_cores)]
            gshape = (sum(s.shape[0] for s in singles),) + singles[0].shape[1:]
            gin.append(jax.make_array_from_single_device_arrays(
                gshape, sh, singles))
        jax.block_until_ready(gin)
        t1 = time.time()
        outs = sharded(*gin, *zouts)
        jax.block_until_ready(outs)
        t2 = time.time()
        shard_arrays = {}
        for i, name in enumerate(out_names):
            shards = sorted(outs[i].addressable_shards,
                            key=lambda s: s.device.id)
            shard_arrays[name] = shards

        def get_one(args):
            name, c = args
            return name, c, np.asarray(shard_arrays[name][c].data)

        res = [dict() for _ in range(n_cores)]
        jobs = [(name, c) for name in out_names for c in range(n_cores)]
        with ThreadPoolExecutor(8) as ex:
            for name, c, arr in ex.map(get_one, jobs):
                res[c][name] = arr
        t3 = time.time()
        timers["put"] = t1 - t0
        timers["exec"] = t2 - t1
        timers["get"] = t3 - t2
        return res

    return run

P = 128
C = 32
E = 64                    # padded feature row elements (256B)
ST = 2048                 # output rows per super-tile
NC8 = 8
HP = 24576                # halo pad (rows)
ZG = 16384                # zero row inserted after every ZG window rows
NG = 7                    # zero-row groups in window
WIN = NG * ZG             # 114688 window rows before zero insertion
WINZ = WIN + NG           # 114695
LIMIT = 32700
SENT = np.int16(-32768)
FP32 = mybir.dt.float32
FP16 = mybir.dt.float16
I16 = mybir.dt.int16
EPS = 1e-5

N0 = 400000
RPC0T = N0 // NC8             # 50000
RPC0P = 51200                 # 25 supers
S0 = RPC0P // ST
XF = NC8 * RPC0P + WIN + 128   # 473216 rows of Xfull

_cache = {}


# ---------------- host-side planning ----------------

def _plan_table(T, kperm, rpc_in_t, rpc_in_p, rpc_out_t, rpc_out_p, m_out_true):
    """T [K, M] original table -> per-core wrapped int16 rel tables + call plan."""
    K = len(kperm)
    n_sup = rpc_out_p // ST
    Tp = np.asarray(T, np.int64)[kperm]
    v = Tp >= 0
    ci = np.clip(np.clip(Tp, 0, None) // rpc_in_t, 0, NC8 - 1)
    g = ci * rpc_in_p + (np.clip(Tp, 0, None) - ci * rpc_in_t)

    NEG = np.int64(1) << 40
    # local window coords per out-core [8, K, rpc_out_p]
    L = np.full((NC8, K, rpc_out_p), NEG, np.int64)
    for c in range(NC8):
        lo = c * rpc_out_t
        hi = min((c + 1) * rpc_out_t, m_out_true)
        n = hi - lo
        if n <= 0:
            continue
        raw = g[:, lo:hi] - c * rpc_in_p + HP
        vv = v[:, lo:hi]
        assert raw[vv].min() >= 0 and raw[vv].max() < WIN, (raw[vv].min(), raw[vv].max())
        lw = raw + raw // ZG
        L[c, :, :n] = np.where(vv, lw, NEG)

    Ls = L.reshape(NC8, K, n_sup, ST)
    if K == 27:
        chunks = [(0, 9), (9, 18), (18, 27)]
    else:
        chunks = [(0, 4), (4, 8)]
    plans = []          # per super: list of (klo, khi, base)
    for s in range(n_sup):
        calls = []
        for (clo, chi) in chunks:
            klo = clo
            while klo < chi:
                khi = chi
                while True:
                    sub = Ls[:, klo:khi, s, :]
                    val = sub[sub < NEG]
                    if val.size == 0:
                        base = 0
                        break
                    base = int(val.min())
                    if int(val.max()) - base < LIMIT or khi == klo + 1:
                        break
                    khi = klo + max(1, (khi - klo) // 2)
                calls.append((klo, khi, base))
                klo = khi
        plans.append(calls)

    # rel16 per core, wrapped [n_sup, 16, K*ST//16]
    rels = []
    for c in range(NC8):
        rel = np.zeros((n_sup, K, ST), np.int16)
        for s in range(n_sup):
            for (klo, khi, base) in plans[s]:
                m = base // (ZG + 1)
                z = m * (ZG + 1) + ZG
                zrel = z - base
                assert 0 <= zrel <= 32767 and z < WINZ
                sub = Ls[c, klo:khi, s, :]
                r = np.where(sub < NEG, sub - base, zrel)
                assert r.min() >= 0 and r.max() <= 32767, (r.min(), r.max())
                rel[s, klo:khi, :] = r.astype(np.int16)
        rels.append(rel.reshape(n_sup, K * ST // 16, 16).transpose(0, 2, 1).copy())
    return rels, plans


def _wrap_check():
    # logical i = k*ST + r must live at wrapped[i % 16, i // 16]
    # rel.reshape(n_sup, K*ST//16, 16).transpose -> [n_sup, 16, K*ST//16]:
    # element (s, i%16, i//16) = rel[s, :, :].flat[i]  (i = k*ST + r)  OK
    pass


# ---------------- program build ----------------

def _build(M1, plans0, plansd, plans1):
    rpc1_t = -(-M1 // NC8)
    rpc1_p = -(-rpc1_t // ST) * ST
    S1 = rpc1_p // ST
    CH0 = RPC0P // 8
    CH1 = rpc1_p // 8

    nc = bacc.Bacc("TRN2", target_bir_lowering=False)
    feat_d = nc.dram_tensor("feat", [RPC0P, 4], FP32, kind="ExternalInput")
    rel0_d = nc.dram_tensor("rel0", [S0, 16, 27 * ST // 16], I16, kind="ExternalInput")
    reld_d = nc.dram_tensor("reld", [S1, 16, 8 * ST // 16], I16, kind="ExternalInput")
    rel1_d = nc.dram_tensor("rel1", [S1, 16, 27 * ST // 16], I16, kind="ExternalInput")
    wts_d = nc.dram_tensor("wts", [170, C, C], FP32, kind="ExternalInput")
    gbt_d = nc.dram_tensor("gbt", [C, 14], FP32, kind="ExternalInput")
    out_d = nc.dram_tensor("out", [rpc1_p, C], FP16, kind="ExternalOutput")

    groups = [list(range(NC8))]

    with tile.TileContext(nc) as tc:
        with (
            tc.tile_pool(name="gb", bufs=3) as gb,
            tc.tile_pool(name="st", bufs=2) as stp,
            tc.tile_pool(name="it", bufs=2) as itp,
            tc.tile_pool(name="sq", bufs=2) as sqp,
            tc.tile_pool(name="sm", bufs=1) as sm,
            tc.tile_pool(name="ps", bufs=2, space="PSUM") as ps,
            tc.tile_pool(name="dram", bufs=1, space="DRAM") as dram,
        ):
            xfull = dram.tile([XF, C], FP32, name="xfull")
            xwin = dram.tile([WINZ, E], FP32, name="xwin")
            xshard = dram.tile([RPC0P, C], FP32, name="xshard")
            rawy = dram.tile([C, RPC0P], FP32, name="rawy")
            x1a = dram.tile([C, rpc1_p], FP32, name="x1a")
            x1b = dram.tile([C, rpc1_p], FP32, name="x1b")
            statin = dram.tile([C, 2], FP32, name="statin")
            statout = dram.tile([C, 2], FP32, name="statout")

            zt = sm.tile([P, 2048], FP32, name="zt")
            nc.vector.memset(zt[:], 0.0)
            tile_zero(nc, xfull[:], zt[:], nc.sync,
                      dangerously_skip_offset_check=True)
            tile_zero(nc, xwin[:], zt[:], nc.sync,
                      dangerously_skip_offset_check=True)
            tile_zero(nc, xshard[:], zt[:], nc.sync,
                      dangerously_skip_offset_check=True)

            gbt_t = sm.tile([C, 14], FP32, name="gbt_t")
            nc.sync.dma_start(gbt_t[:], gbt_d[:])

            # initial features into xshard[:, 0:4]
            nc.sync.dma_start(xshard[:, 0:4], feat_d[:])

            pid = nc.sync.partition_id()

            layers = [
                # (tag, rel_d, K, plans, n_sup, rpc_in, rpc_out, w_off, gb_i,
                #  res_in, res_out, final, inv_n_idx)
                ("s1", rel0_d, 27, plans0, S0, RPC0P, RPC0P, 0, 0, None, None, False),
                ("s2", rel0_d, 27, plans0, S0, RPC0P, RPC0P, 27, 1, None, None, False),
                ("dn", reld_d, 8, plansd, S1, RPC0P, rpc1_p, 54, 2, None, x1a, False),
                ("ra", rel1_d, 27, plans1, S1, rpc1_p, rpc1_p, 62, 3, None, None, False),
                ("rb", rel1_d, 27, plans1, S1, rpc1_p, rpc1_p, 89, 4, x1a, x1b, False),
                ("rc", rel1_d, 27, plans1, S1, rpc1_p, rpc1_p, 116, 5, None, None, False),
                ("rd", rel1_d, 27, plans1, S1, rpc1_p, rpc1_p, 143, 6, x1b, None, True),
            ]
            inv_ns = [1.0 / N0, 1.0 / N0, 1.0 / M1, 1.0 / M1, 1.0 / M1,
                      1.0 / M1, 1.0 / M1]

            import os
            nlay = int(os.environ.get("KLAYERS", "7"))
            kstage = int(os.environ.get("KSTAGE", "7"))
            layers = layers[:nlay]

            for (tag, rel_d, K, plans, n_sup, rpc_in, rpc_out, w_off, gb_i,
                 res_in, res_out, final) in layers:
                inv_n = inv_ns[gb_i]
                # --- AllGather previous output, copy halo window ---
                nc.gpsimd.collective_compute(
                    "AllGather", mybir.AluOpType.bypass,
                    replica_groups=groups,
                    ins=[xshard[0:rpc_in, :]],
                    outs=[xfull[HP:HP + NC8 * rpc_in, :]],
                )
                for g7 in range(NG):
                    nc.sync.dma_start(
                        xwin[g7 * (ZG + 1):g7 * (ZG + 1) + ZG, 0:C],
                        xfull[DynSlice(pid * rpc_in + g7 * ZG, ZG), :])

                # --- weights [32ci, K, 32co] replicated over 4 groups ---
                wrep = sm.tile([P, K, C], FP32, name="wrep", tag="wrep")
                for g4 in range(4):
                    nc.sync.dma_start(
                        wrep[32 * g4:32 * g4 + 32, :, :],
                        wts_d[w_off:w_off + K].rearrange("k i o -> i k o"))

                stS = sm.tile([C, n_sup * 4], FP32, name="stS", tag="stS")
                stQ = sm.tile([C, n_sup * 4], FP32, name="stQ", tag="stQ")

                if K == 27:
                    chunks = [(0, 9), (9, 18), (18, 27)]
                else:
                    chunks = [(0, 4), (4, 8)]

                # --- pass 1: conv + stats ---
                for s in range(n_sup):
                    if kstage < 2:
                        break
                    idxt = itp.tile([P, K * ST // 16], I16, name="idxt", tag="it")
                    for g8 in range(8):
                        nc.sync.dma_start(idxt[16 * g8:16 * g8 + 16, :],
                                          rel_d[s, :, :])
                    accs = [ps.tile([C, 16, C], FP32, name=f"acc{g4}", tag=f"acc{g4}")
                            for g4 in range(4)]
                    calls = {}
                    for (klo, khi, base) in plans[s]:
                        calls[klo] = (khi, base)
                    for (clo, chi) in chunks:
                        gath = gb.tile([P, chi - clo, 16, E], FP32,
                                       name="gath", tag="big")
                        if kstage < 3:
                            nc.vector.memset(gath[:], 0.0)
                        kgmax = int(os.environ.get("KGMAX", "1"))
                        klo = clo
                        while klo < chi:
                            khi, base = calls[klo]
                            hi = min(base + 32768, WINZ)
                            for k0 in range(klo, khi, kgmax):
                                k1 = min(k0 + kgmax, khi)
                                nidx = (k1 - k0) * ST
                                if kstage >= 3:
                                    nc.gpsimd.dma_gather(
                                        out_ap=gath[:, k0 - clo:k1 - clo, :, :].rearrange(
                                            "p a b e -> p (a b) e"),
                                        in_ap=xwin[base:hi, :],
                                        idxs_ap=idxt[:, k0 * P:k1 * P],
                                        num_idxs=nidx,
                                        num_idxs_reg=nidx,
                                        elem_size=E,
                                        single_packet=False,
                                    )
                            klo = khi
                        strt = stp.tile([P, chi - clo, 16, C], FP32,
                                        name="strt", tag="st")
                        if kstage >= 4:
                            nc.vector.transpose(strt[:], gath[:, :, :, 0:C])
                        else:
                            nc.vector.memset(strt[:], 0.0)
                        for k in range(clo, chi):
                            for g4 in range(4):
                                nc.tensor.matmul(
                                    accs[g4][:, :, :],
                                    wrep[32 * g4:32 * g4 + 32, k, :],
                                    strt[32 * g4:32 * g4 + 32, k - clo, :, :],
                                    start=(k == 0), stop=(k == K - 1),
                                    tile_position=(32 * g4, 0),
                                )
                    for g4 in range(4):
                        col = rawy[:, s * ST:(s + 1) * ST].rearrange(
                            "c (q x) -> c q x", x=P)[:, :, 32 * g4:32 * g4 + 32]
                        acc_sb = sqp.tile([C, 16, C], FP32, name="acc_sb",
                                          tag="acc_sb")
                        nc.scalar.activation(acc_sb[:], accs[g4][:],
                                             mybir.ActivationFunctionType.Copy)
                        nc.sync.dma_start(col, acc_sb[:])
                        nc.vector.tensor_reduce(
                            stS[:, s * 4 + g4:s * 4 + g4 + 1],
                            acc_sb[:].rearrange("c q x -> c (q x)"),
                            axis=mybir.AxisListType.X, op=mybir.AluOpType.add)
                        sq = sqp.tile([C, 16, C], FP32, name="sq", tag="sq")
                        nc.vector.tensor_tensor(out=sq[:], in0=acc_sb[:],
                                                in1=acc_sb[:],
                                                op=mybir.AluOpType.mult)
                        nc.vector.tensor_reduce(
                            stQ[:, s * 4 + g4:s * 4 + g4 + 1],
                            sq[:].rearrange("c q x -> c (q x)"),
                            axis=mybir.AxisListType.X, op=mybir.AluOpType.add)

                # --- BN stats: fold + AllReduce + coefficients ---
                loc = sm.tile([C, 2], FP32, name="loc", tag="loc")
                nc.vector.tensor_reduce(loc[:, 0:1], stS[:],
                                        axis=mybir.AxisListType.X,
                                        op=mybir.AluOpType.add)
                nc.vector.tensor_reduce(loc[:, 1:2], stQ[:],
                                        axis=mybir.AxisListType.X,
                                        op=mybir.AluOpType.add)
                nc.sync.dma_start(statin[:], loc[:])
                nc.gpsimd.collective_compute(
                    "AllReduce", mybir.AluOpType.add,
                    replica_groups=groups,
                    ins=[statin.opt()], outs=[statout.opt()],
                )
                tot = sm.tile([C, 2], FP32, name="tot", tag="tot")
                nc.sync.dma_start(tot[:], statout[:])
                mu = sm.tile([C, 1], FP32, name="mu", tag="mu")
                nc.vector.tensor_scalar_mul(mu[:], tot[:, 0:1], float(inv_n))
                var = sm.tile([C, 1], FP32, name="var", tag="var")
                nc.vector.tensor_scalar_mul(var[:], tot[:, 1:2], float(inv_n))
                mu2 = sm.tile([C, 1], FP32, name="mu2", tag="mu2")
                nc.vector.tensor_tensor(out=mu2[:], in0=mu[:], in1=mu[:],
                                        op=mybir.AluOpType.mult)
                nc.vector.tensor_tensor(out=var[:], in0=var[:], in1=mu2[:],
                                        op=mybir.AluOpType.subtract)
                nc.vector.tensor_scalar_add(var[:], var[:], EPS)
                std = sm.tile([C, 1], FP32, name="std", tag="std")
                nc.scalar.sqrt(std[:], var[:])
                rstd = sm.tile([C, 1], FP32, name="rstd", tag="rstd")
                nc.vector.reciprocal(rstd[:], std[:])
                s_v = sm.tile([C, 1], FP32, name="s_v", tag="s_v")
                b_v = sm.tile([C, 1], FP32, name="b_v", tag="b_v")
                nc.vector.tensor_tensor(out=s_v[:], in0=gbt_t[:, gb_i:gb_i + 1],
                                        in1=rstd[:], op=mybir.AluOpType.mult)
                mus = sm.tile([C, 1], FP32, name="mus", tag="mus")
                nc.vector.tensor_tensor(out=mus[:], in0=mu[:], in1=s_v[:],
                                        op=mybir.AluOpType.mult)
                nc.vector.tensor_tensor(out=b_v[:], in0=gbt_t[:, 7 + gb_i:8 + gb_i],
                                        in1=mus[:], op=mybir.AluOpType.subtract)

                # --- pass 2: affine (+res) + relu + transpose + writeout ---
                CH = rpc_out // 8
                for j in range(8):
                    sl = slice(j * CH, (j + 1) * CH)
                    raw = gb.tile([C, CH], FP32, name="p2raw", tag="big")
                    nc.sync.dma_start(raw[:], rawy[:, sl])
                    nc.vector.tensor_scalar(
                        out=raw[:], in0=raw[:], scalar1=s_v[:], scalar2=b_v[:],
                        op0=mybir.AluOpType.mult, op1=mybir.AluOpType.add)
                    if res_in is not None:
                        x1t = gb.tile([C, CH], FP32, name="p2x1", tag="big")
                        nc.sync.dma_start(x1t[:], res_in[:, sl])
                        nc.vector.tensor_tensor(out=raw[:], in0=raw[:],
                                                in1=x1t[:],
                                                op=mybir.AluOpType.add)
                    nc.scalar.activation(raw[:], raw[:],
                                         mybir.ActivationFunctionType.Relu)
                    if res_out is not None:
                        nc.sync.dma_start(res_out[:, sl], raw[:])
                    trt = gb.tile([C, CH], FP32, name="p2tr", tag="big")
                    nc.vector.transpose(trt[:], raw[:])
                    if final:
                        trh = gb.tile([C, CH], FP16, name="p2trh", tag="big")
                        nc.vector.tensor_copy(trh[:], trt[:])
                        dstv = out_d[sl, :].rearrange("(b j) c -> j b c", j=C)
                        nc.sync.dma_start(
                            dstv, trh[:, :].rearrange("j (b c) -> j b c", c=C))
                    else:
                        dstv = xshard[sl, :].rearrange("(b j) c -> j b c", j=C)
                        nc.sync.dma_start(
                            dstv, trt[:, :].rearrange("j (b c) -> j b c", c=C))
    nc.compile()
    return nc


# ---------------- host orchestration ----------------

def kernel(voxel_features, W_stem1, W_stem2, W_down, W_r1a, W_r1b, W_r2a, W_r2b,
           gammas, betas, nbr0, down1, nbr1):
    import time
    kernel.compile_s = 0.0
    kernel.host_s = 0.0
    t0 = time.time()

    vf = np.asarray(voxel_features, np.float32)
    nbr0 = np.asarray(nbr0, np.int64)
    down1 = np.asarray(down1, np.int64)
    nbr1 = np.asarray(nbr1, np.int64)
    M1 = nbr1.shape[1]
    rpc1_t = -(-M1 // NC8)
    rpc1_p = -(-rpc1_t // ST) * ST

    kperm27 = [k for dz in range(3) for k in range(27) if k % 3 == dz]
    kperm8 = [0, 2, 4, 6, 1, 3, 5, 7]

    rels0, plans0 = _plan_table(nbr0, kperm27, RPC0T, RPC0P, RPC0T, RPC0P, N0)
    relsd, plansd = _plan_table(down1, kperm8, RPC0T, RPC0P, rpc1_t, rpc1_p, M1)
    rels1, plans1 = _plan_table(nbr1, kperm27, rpc1_t, rpc1_p, rpc1_t, rpc1_p, M1)

    # weights: [170, 32, 32] k-permuted per layer; stem1 padded 4->32
    Ws = []
    w1 = np.zeros((27, C, C), np.float32)
    w1[:, 0:4, :] = np.asarray(W_stem1, np.float32)
    Ws.append(w1[kperm27])
    Ws.append(np.asarray(W_stem2, np.float32)[kperm27])
    Ws.append(np.asarray(W_down, np.float32)[kperm8])
    for W in (W_r1a, W_r1b, W_r2a, W_r2b):
        Ws.append(np.asarray(W, np.float32)[kperm27])
    wts = np.concatenate(Ws, 0)
    assert wts.shape[0] == 170

    gbt = np.zeros((C, 14), np.float32)
    gbt[:, 0:7] = np.asarray(gammas, np.float32).T
    gbt[:, 7:14] = np.asarray(betas, np.float32).T

    key = (M1, repr(plans0), repr(plansd), repr(plans1))
    if key not in _cache:
        t = time.time()
        prog = _build(M1, plans0, plansd, plans1)
        runner = _make_runner(prog, NC8)
        # warmup with zeros
        zmaps = []
        for c in range(NC8):
            zmaps.append({
                "feat": np.zeros((RPC0P, 4), np.float32),
                "rel0": np.zeros_like(rels0[c]),
                "reld": np.zeros_like(relsd[c]),
                "rel1": np.zeros_like(rels1[c]),
                "wts": np.zeros((170, C, C), np.float32),
                "gbt": np.zeros((C, 14), np.float32),
            })
        runner(zmaps, {})
        kernel.compile_s += time.time() - t
        _cache[key] = runner
    runner = _cache[key]

    in_maps = []
    for c in range(NC8):
        fpad = np.zeros((RPC0P, 4), np.float32)
        n = min(RPC0T, N0 - c * RPC0T)
        fpad[:n] = vf[c * RPC0T:c * RPC0T + n]
        in_maps.append({
            "feat": fpad,
            "rel0": rels0[c],
            "reld": relsd[c],
            "rel1": rels1[c],
            "wts": wts,
            "gbt": gbt,
        })
    kernel.host_s += time.time() - t0

    t = time.time()
    timers = {}
    results = runner(in_maps, timers)
    kernel.exec_s = time.time() - t
    kernel.timers = timers

    t = time.time()
    out = np.empty((M1, C), np.float32)
    for c in range(NC8):
        lo = c * rpc1_t
        hi = min((c + 1) * rpc1_t, M1)
        out[lo:hi] = results[c]["out"][:hi - lo].astype(np.float32)
    kernel.host_s += time.time() - t
    return out


kernel.exec_s = 0.0
kernel.compile_s = 0.0
kernel.host_s = 0.0
